# revision 9
# baseline (speedup 1.0000x reference)
# Block-circulant linear kernel for Trainium2 (Bass/Tile), 8-core SPMD.
#
# y[b, 16m+p] = sum_{n,q} blocks[(m-n)%512, p, q] * x[b, 16n+q]
#
# Strategy: shard the output block axis m across 8 cores (64 block-rows each).
# Per core, store a doubled+shifted "BIGQ" layout of blocks in SBUF:
#     BIGQ[(ni,q), u*16+p] = blocks[(m0 + u - ni) % 512, p, q]
# so that EVERY 128x128 weight tile of the implied 8192x8192 circulant matrix
# is a contiguous 128-column slice of BIGQ (the circulant gather becomes pure
# addressing). All (m_tile t, n_chunk c) pairs with the same diagonal offset
# d = t - c share one stationary tile, so the whole per-core compute is 71
# accumulating matmuls into a single PSUM bank [128 mp, 8 t x 32 b].
import numpy as np

B = 32
NB = 512          # number of 16x16 blocks
NCORES = 8
MBLK = NB // NCORES   # 64 output block-rows per core
W = 576               # BIGQ window width (in u units of 16 columns)
ND = 71               # diagonal offsets d in [-63, 7]

_cached_nc = None
_last_results = None  # BassKernelResults of the most recent run (for profiling)


def _build_program():
    import concourse.bacc as bacc
    import concourse.mybir as mybir
    import concourse.tile as tile

    # Bacc (not plain Bass): its compile() pipeline splits multi-wait
    # instructions into EventSemaphore preludes (HW allows 1 wait/inst).
    nc = bacc.Bacc("TRN2", target_bir_lowering=False, debug=False, num_devices=NCORES)
    f32 = mybir.dt.float32
    # single concatenated input: [xt (2048 cols) | bigq (W*16 cols)] so the
    # first DMA chunk (xt + first bigq slab) is ONE transfer -> the first
    # matmul needs only one semaphore wait (walrus allows max 1 per matmul).
    XCOLS = 2048
    TOT = XCOLS + W * 16
    xb_d = nc.declare_dram_parameter("xtbq", [128, TOT], f32, isOutput=False)
    out_d = nc.declare_dram_parameter("out", [128, 256], f32, isOutput=True)

    # chunk boundaries (columns): first chunk = xt + first bigq slab, then 6
    # more bigq chunks (boundaries on the 128-col grid so no matmul weight
    # window crosses a chunk). 7 input DMAs + 1 output DMA = 8 total, one per
    # DMAHW semaphore lane (lane reuse would add a second sync wait, which
    # walrus rejects).
    bq_blk = [0, 9, 19, 29, 40, 51, 62, 72]  # bigq boundaries in 128-col units
    bounds = [0] + [XCOLS + 128 * b for b in bq_blk[1:]]

    with tile.TileContext(nc) as tc:
        with (
            tc.tile_pool(name="data", bufs=1) as data_pool,
            tc.tile_pool(name="psum", bufs=1, space="PSUM") as psum_pool,
        ):
            xb = data_pool.tile([128, TOT], f32)
            out_sb = data_pool.tile([128, 256], f32)
            acc = psum_pool.tile([128, 256], f32)

            xt = xb[:, 0:XCOLS]
            bq = xb[:, XCOLS:TOT]

            for ci in range(7):
                nc.sync.dma_start(
                    xb[:, bounds[ci]:bounds[ci + 1]],
                    xb_d[:, bounds[ci]:bounds[ci + 1]],
                )

            # d = t - c diagonal; stationary tile = BIGQ columns [16*u0, 16*u0+128)
            # with u0 = 8*d + 512 = 8*i + 8 for i = 0..70 (d = i - 63).
            for i in range(ND):
                d = i - 63
                u0 = 8 * i + 8
                t_lo = max(0, d)
                t_hi = min(7, 63 + d)
                c_lo = t_lo - d
                nt = t_hi - t_lo + 1
                nc.tensor.matmul(
                    acc[:, 32 * t_lo: 32 * (t_lo + nt)],
                    bq[:, 16 * u0: 16 * u0 + 128],
                    xt[:, 32 * c_lo: 32 * (c_lo + nt)],
                    start=(i == 0),   # clears the whole PSUM bank
                    stop=(i == ND - 1),
                )

            nc.vector.tensor_copy(out_sb[:], acc[:])
            nc.sync.dma_start(out_d[:], out_sb[:])
    nc.compile()
    return nc


def _get_program():
    global _cached_nc
    if _cached_nc is None:
        _cached_nc = _build_program()
    return _cached_nc


def _prep_inputs(x, blocks):
    """Host-side layout prep (pure numpy reshuffles of the small inputs)."""
    x = np.ascontiguousarray(np.asarray(x), dtype=np.float32)
    blocks = np.ascontiguousarray(np.asarray(blocks), dtype=np.float32)
    # xt[(ni*16+q), c*32+b] = x[b, 128c + 16ni + q]
    xt = np.ascontiguousarray(
        x.T.reshape(64, 128, 32).transpose(1, 0, 2).reshape(128, 2048)
    )
    u = np.arange(W)
    ni = np.arange(8)
    in_maps = []
    for k in range(NCORES):
        m0 = k * MBLK
        idx = (m0 + u[None, :] - ni[:, None]) % NB        # [8, W]
        bigq = blocks[idx]                                 # [8, W, p, q]
        bigq = bigq.transpose(0, 3, 1, 2).reshape(128, W * 16)  # [(ni,q), (u,p)]
        xtbq = np.ascontiguousarray(np.concatenate([xt, bigq], axis=1))
        in_maps.append({"xtbq": xtbq})
    return in_maps


def _assemble(results):
    y = np.empty((B, NB * 16), dtype=np.float32)
    for k in range(NCORES):
        o = results[k]["out"]  # [128 (mi,p), 256 (t,b)]
        y[:, 1024 * k: 1024 * (k + 1)] = (
            o.reshape(128, 8, 32).transpose(2, 1, 0).reshape(32, 1024)
        )
    return y


def kernel(x, blocks):
    global _last_results
    from concourse.bass_utils import run_bass_kernel_spmd

    nc = _get_program()
    in_maps = _prep_inputs(x, blocks)
    res = run_bass_kernel_spmd(nc, in_maps, list(range(NCORES)))
    _last_results = res
    return _assemble(res.results)


# revision 10
# speedup vs baseline: 1.9366x; 1.9366x over previous
# Block-circulant linear kernel for Trainium2 (Bass/Tile), 8-core SPMD.
#
# y[b, 16m+p] = sum_{n,q} blocks[(m-n)%512, p, q] * x[b, 16n+q]
#
# Strategy: shard the output block axis m across 8 cores (64 block-rows each).
# Per core, store a doubled+shifted "BIGQ" layout of blocks in SBUF:
#     BIGQ[(ni,q), u*16+p] = blocks[(m0 + u - ni) % 512, p, q]
# so that EVERY 128x128 weight tile of the implied 8192x8192 circulant matrix
# is a contiguous 128-column slice of BIGQ (the circulant gather becomes pure
# addressing). All (m_tile t, n_chunk c) pairs with the same diagonal offset
# d = t - c share one stationary tile, so the whole per-core compute is 71
# accumulating matmuls into a single PSUM bank [128 mp, 8 t x 32 b].
#
# The xt layout is reversed (c' = 63 - c) and the psum t axis flipped
# (t' = 7 - t) so both the weight stream (BIGQ u ascending) and the moving
# stream (xt c' ascending) are consumed in DMA arrival order.
import numpy as np

B = 32
NB = 512          # number of 16x16 blocks
NCORES = 8
MBLK = NB // NCORES   # 64 output block-rows per core
W = 576               # BIGQ window width (in u units of 16 columns)
ND = 71               # diagonal offsets d in [-63, 7]

# matmul operand dtype: "float32" (exact, 4 cyc/row), "float32r" (1 cyc/row
# at N>=256), "bfloat16" (1 cyc/row + fast weight load, ~1e-3 error)
DTYPE = "float32"

_cached = {}
_last_results = None  # BassKernelResults of the most recent run (for profiling)


def _np_dtype(name):
    if name == "bfloat16":
        import ml_dtypes

        return ml_dtypes.bfloat16
    return np.float32


def _build_program(dt_name):
    import concourse.bacc as bacc
    import concourse.mybir as mybir
    import concourse.tile as tile

    dt = getattr(mybir.dt, dt_name)
    f32 = mybir.dt.float32

    # Bacc (not plain Bass): its compile() pipeline splits multi-wait
    # instructions into EventSemaphore preludes (HW allows 1 wait/inst).
    nc = bacc.Bacc("TRN2", target_bir_lowering=False, debug=False, num_devices=NCORES)
    xt_d = nc.declare_dram_parameter("xt", [128, 2048], dt, isOutput=False)
    bq_d = nc.declare_dram_parameter("bigq", [128, W * 16], dt, isOutput=False)
    out_d = nc.declare_dram_parameter("out", [128, 256], f32, isOutput=True)

    NCH = 8
    csz = (W * 16) // NCH  # 1152 bigq cols per chunk

    with tile.TileContext(nc) as tc:
        with (
            tc.tile_pool(name="data", bufs=1) as data_pool,
            tc.tile_pool(name="psum", bufs=1, space="PSUM") as psum_pool,
        ):
            xt = data_pool.tile([128, 2048], dt)
            bq = data_pool.tile([128, W * 16], dt)
            out_sb = data_pool.tile([128, 256], f32)
            acc = psum_pool.tile([128, 256], f32)

            # interleave the streams in consumption order: first xt half +
            # first bigq chunks feed the earliest matmuls.
            nc.sync.dma_start(xt[:, 0:1024], xt_d[:, 0:1024])
            for ci in range(NCH // 2):
                nc.sync.dma_start(
                    bq[:, ci * csz:(ci + 1) * csz], bq_d[:, ci * csz:(ci + 1) * csz]
                )
            nc.sync.dma_start(xt[:, 1024:2048], xt_d[:, 1024:2048])
            for ci in range(NCH // 2, NCH):
                nc.sync.dma_start(
                    bq[:, ci * csz:(ci + 1) * csz], bq_d[:, ci * csz:(ci + 1) * csz]
                )

            # d = t - c diagonal; stationary tile = BIGQ columns [16*u0, 16*u0+128)
            # with u0 = 8*i + 8 for i = 0..70 (d = i - 63).
            for i in range(ND):
                d = i - 63
                u0 = 8 * i + 8
                t_lo = max(0, d)
                t_hi = min(7, 63 + d)
                nt = t_hi - t_lo + 1
                tp_lo = 7 - t_hi           # flipped psum tile index
                cp_lo = 63 + d - t_hi      # reversed xt chunk index
                nc.tensor.matmul(
                    acc[:, 32 * tp_lo: 32 * (tp_lo + nt)],
                    bq[:, 16 * u0: 16 * u0 + 128],
                    xt[:, 32 * cp_lo: 32 * (cp_lo + nt)],
                    start=(i == 0),   # clears the whole PSUM bank
                    stop=(i == ND - 1),
                )

            nc.vector.tensor_copy(out_sb[:], acc[:])
            nc.sync.dma_start(out_d[:], out_sb[:])
    nc.compile()
    return nc


def _get_program(dt_name):
    if dt_name not in _cached:
        _cached[dt_name] = _build_program(dt_name)
    return _cached[dt_name]


def _prep_inputs(x, blocks, dt_name):
    """Host-side layout prep (pure numpy reshuffles of the small inputs)."""
    npdt = _np_dtype(dt_name)
    x = np.ascontiguousarray(np.asarray(x), dtype=np.float32)
    blocks = np.ascontiguousarray(np.asarray(blocks), dtype=np.float32)
    # xt[(ni*16+q), c*32+b] = x[b, 128c + 16ni + q], then reverse c (c'=63-c)
    xt = x.T.reshape(64, 128, 32).transpose(1, 0, 2)[:, ::-1, :].reshape(128, 2048)
    xt = np.ascontiguousarray(xt.astype(npdt))
    u = np.arange(W)
    ni = np.arange(8)
    in_maps = []
    for k in range(NCORES):
        m0 = k * MBLK
        idx = (m0 + u[None, :] - ni[:, None]) % NB        # [8, W]
        bigq = blocks[idx]                                 # [8, W, p, q]
        bigq = bigq.transpose(0, 3, 1, 2).reshape(128, W * 16)  # [(ni,q), (u,p)]
        in_maps.append({"xt": xt, "bigq": np.ascontiguousarray(bigq.astype(npdt))})
    return in_maps


def _assemble(results):
    y = np.empty((B, NB * 16), dtype=np.float32)
    for k in range(NCORES):
        o = np.asarray(results[k]["out"])  # [128 (mi,p), 256 (t',b)], t = 7-t'
        y[:, 1024 * k: 1024 * (k + 1)] = (
            o.reshape(128, 8, 32)[:, ::-1, :].transpose(2, 1, 0).reshape(32, 1024)
        )
    return y


def kernel(x, blocks):
    global _last_results
    from concourse.bass_utils import run_bass_kernel_spmd

    nc = _get_program(DTYPE)
    in_maps = _prep_inputs(x, blocks, DTYPE)
    res = run_bass_kernel_spmd(nc, in_maps, list(range(NCORES)))
    _last_results = res
    return _assemble(res.results)
